# revision 20
# baseline (speedup 1.0000x reference)
"""Multi-head attention (B=2, L=2048, DIM=2048, H=16) on 8 TRN2 NeuronCores.

Sharding: data-parallel over batch (2) x tensor-parallel over head groups (4).
Core c handles batch c//4, heads [4*(c%4), 4*(c%4)+4).

Host-side prep (not on the device critical path):
  - inputs are pre-transposed to feature-major layout and cast to bf16, so
    the device does zero PE transposes;
  - the multiplicative per-query mask is folded into xq, and only rows with
    nonzero mask are shipped (packed); masked rows' output is the uniform
    softmax = column-mean of v, reconstructed on the host.

Device math per core (head h, dh=128), all matmuls bf16 with fp32 PSUM:
  K_T[d, k] / V[k, j] from xvt chunks; Qt[d, q] from packed xqt chunks.
  S[k, q] = K_T . Qt;  E = exp(S / sqrt(128)) in bf16 (Act engine)
  attn[q, 0:128] / den[q, 128] from one matmul with a ones-column
  appended to V; out = attn * (1/den) per partition.
The q loop software-pipelines: Q-projection of chunk qc+1 and AV of head
h-1 are interleaved with scores/exp of head h so the PE never waits on
the Act engine's exp throughput.
"""

import sys

for p in ("/opt/trn_rl_repo", "/opt/pypackages"):
    if p not in sys.path:
        sys.path.insert(0, p)

import contextlib

import ml_dtypes
import numpy as np

import concourse.bacc as bacc
import concourse.mybir as mybir
import concourse.tile as tile
from concourse.bass_utils import run_bass_kernel_spmd

N_CORES = 8
B, L, DIM, H = 2, 2048, 2048, 16
JB = DIM // 4          # 512 output features per core (4 heads)
DH = 128               # head dim
NH = 4                 # heads per core
NDC = DIM // 128       # 16 contraction chunks
QC = 512               # q-chunk width (one PSUM bank of fp32)
SCALE = 1.0 / np.sqrt(DH)

F32 = mybir.dt.float32
BF16 = mybir.dt.bfloat16
BF16_NP = ml_dtypes.bfloat16


def build_nc(NP, reps=1):
    """DRAM layouts are host-pretiled so every stage is ONE batched DMA:
    x tensors as [128, NDC, cols] (partition-major dc tiling of x^T),
    W tensors as [128, NDC*JB] (partition-major dc tiling of W^T).
    reps>1 unrolls the whole kernel back-to-back inside one program
    (timing only: the axon bass_exec path allows one custom call per
    dispatch, so on-device unrolling is the only way to amortize
    dispatch overhead out of a measurement)."""
    nc = bacc.Bacc("TRN2", target_bir_lowering=False, debug=False,
                   num_devices=N_CORES)
    xqt = nc.dram_tensor("xqt", [128, NDC, NP], BF16,
                         kind="ExternalInput").ap()
    xvt = nc.dram_tensor("xvt", [128, NDC, L], BF16,
                         kind="ExternalInput").ap()
    wqt = nc.dram_tensor("wqt", [128, NDC * JB], BF16,
                         kind="ExternalInput").ap()
    wkt = nc.dram_tensor("wkt", [128, NDC * JB], BF16,
                         kind="ExternalInput").ap()
    wvt = nc.dram_tensor("wvt", [128, NDC * JB], BF16,
                         kind="ExternalInput").ap()
    out = nc.dram_tensor("out", [NP, JB], F32, kind="ExternalOutput").ap()

    with tile.TileContext(nc) as tc:
        for _ in range(reps):
            build_kernel(nc, tc, NP, xqt, xvt, wqt, wkt, wvt, out)
    nc.compile()
    return nc


def build_kernel(nc, tc, NP, xqt, xvt, wqt, wkt, wvt, out):
    NQC = NP // QC
    NKT = L // 128      # 16 k tiles

    with contextlib.ExitStack() as octx:
        w_pool = octx.enter_context(tc.tile_pool(name="w", bufs=1))
        kt_pool = octx.enter_context(tc.tile_pool(name="kt", bufs=1))
        v_pool = octx.enter_context(tc.tile_pool(name="v", bufs=1))
        xt_pool = octx.enter_context(tc.tile_pool(name="xt", bufs=2))
        qt_pool = octx.enter_context(tc.tile_pool(name="qt", bufs=2))
        pacc = octx.enter_context(
            tc.tile_pool(name="pacc", bufs=2, space="PSUM"))

        wq_a = w_pool.tile([128, NDC * JB], BF16, tag="wq", name="wq_a")
        wk_a = w_pool.tile([128, NDC * JB], BF16, tag="wk", name="wk_a")
        wv_a = w_pool.tile([128, NDC * JB], BF16, tag="wv", name="wv_a")
        K_T = [kt_pool.tile([128, L], BF16, tag=f"k{h}", name=f"kT{h}")
               for h in range(NH)]
        V = [v_pool.tile([128, NH * 129], BF16, tag=f"v{t}", name=f"vS{t}")
             for t in range(NKT)]

        for t in range(NKT):
            for h in range(NH):
                nc.vector.memset(V[t][:, h * 129 + 128:h * 129 + 129], 1.0)

        def load_x_chunk(x_dram, c0, w, splits=1):
            xt = xt_pool.tile([128, NDC * w], BF16, tag="xt", name="xt")
            if splits == 1:
                nc.sync.dma_start(xt[:], x_dram[:, :, c0:c0 + w])
            else:
                hd = NDC // splits
                for s in range(splits):
                    nc.sync.dma_start(
                        xt[:, s * hd * w:(s + 1) * hd * w],
                        x_dram[:, s * hd:(s + 1) * hd, c0:c0 + w])
            return xt

        def emit_qproj_half(j, xt, half, acc):
            """Half of head-j's Q projection (dc 8*half..8*half+8).
            Returns (acc, qt): qt is the drained bf16 tile after half 1."""
            if half == 0:
                acc = pacc.tile([128, QC], F32, tag="pacc", name="qacc")
            for dc in range(half * (NDC // 2), (half + 1) * (NDC // 2)):
                nc.tensor.matmul(
                    acc[:],
                    wq_a[:, dc * JB + j * 128:dc * JB + (j + 1) * 128],
                    xt[:, dc * QC:(dc + 1) * QC],
                    start=(dc == 0), stop=(dc == NDC - 1))
            if half == 0:
                return acc, None
            qt = qt_pool.tile([128, QC], BF16, tag=f"qt{j}", name=f"qt{j}")
            nc.vector.tensor_copy(qt[:], acc[:])
            return acc, qt

        def emit_qproj_j(j, xt):
            acc, _ = emit_qproj_half(j, xt, 0, None)
            _, qt = emit_qproj_half(j, xt, 1, acc)
            return qt

        # ---- prologue: Q projection of chunk 0 ----
        # DMA issue order front-loads exactly what the PE needs first;
        # wq/xq (and wk/xv) land as interleaved halves so the first
        # accumulation pass can start after ~1/4 of the transfer.
        qt_cur = None
        if NQC > 0:
            xt_q = xt_pool.tile([128, NDC * QC], BF16, tag="xt", name="xt")
            NS = 8
            wd, xd = NDC * JB // NS, NDC // NS
            for s in range(NS):
                nc.sync.dma_start(wq_a[:, s * wd:(s + 1) * wd],
                                  wqt[:, s * wd:(s + 1) * wd])
                nc.sync.dma_start(xt_q[:, s * xd * QC:(s + 1) * xd * QC],
                                  xqt[:, s * xd:(s + 1) * xd, 0:QC])
            qt_cur = [emit_qproj_j(j, xt_q) for j in range(NH)]
        else:
            nc.sync.dma_start(wq_a[:], wqt[:, :])

        # ---- K/V projections ----
        HW_ = NDC * JB // 2
        nc.sync.dma_start(wk_a[:, :HW_], wkt[:, :HW_])
        xt_v = load_x_chunk(xvt, 0, QC, splits=2)
        nc.sync.dma_start(wk_a[:, HW_:], wkt[:, HW_:])
        nc.sync.dma_start(wv_a[:], wvt[:, :])
        for lb in range(L // QC):
            for j in range(NH):
                acc = pacc.tile([128, QC], F32, tag="pacc", name="kacc")
                for dc in range(NDC):
                    nc.tensor.matmul(
                        acc[:],
                        wk_a[:, dc * JB + j * 128:dc * JB + (j + 1) * 128],
                        xt_v[:, dc * QC:(dc + 1) * QC],
                        start=(dc == 0), stop=(dc == NDC - 1))
                nc.vector.tensor_copy(K_T[j][:, lb * QC:(lb + 1) * QC],
                                      acc[:])
            for kt in range(QC // 128):
                acc = pacc.tile([128, JB], F32, tag="pacc", name="vacc")
                for dc in range(NDC):
                    nc.tensor.matmul(
                        acc[:],
                        xt_v[:, dc * QC + kt * 128:dc * QC + (kt + 1) * 128],
                        wv_a[:, dc * JB:(dc + 1) * JB],
                        start=(dc == 0), stop=(dc == NDC - 1))
                kc = lb * (QC // 128) + kt
                for h in range(NH):
                    nc.vector.tensor_copy(
                        V[kc][:, h * 129:h * 129 + 128],
                        acc[:, h * 128:(h + 1) * 128])
            if lb + 1 < L // QC:
                xt_v = load_x_chunk(xvt, (lb + 1) * QC, QC)

        # ---- attention over packed q chunks (sw-pipelined) ----
        with tc.tile_pool(name="et", bufs=2) as et_pool, \
             tc.tile_pool(name="s_ps", bufs=2, space="PSUM") as sps, \
             tc.tile_pool(name="a_ps", bufs=2, space="PSUM") as aps, \
             tc.tile_pool(name="o_sb", bufs=2) as osb, \
             tc.tile_pool(name="r_sb", bufs=4) as rsb:

            def emit_av(h, et_h, ots, qc_dma=None):
                for qs in range(QC // 128):
                    a = aps.tile([128, 129], F32, tag="a", name="a")
                    for kc in range(NKT):
                        nc.tensor.matmul(
                            a[:],
                            et_h[kc // 2][:, (kc % 2) * QC
                                          + qs * 128:(kc % 2) * QC
                                          + (qs + 1) * 128],
                            V[kc][:, h * 129:(h + 1) * 129],
                            start=(kc == 0), stop=(kc == NKT - 1))
                    rec = rsb.tile([128, 1], F32, tag="rec", name="rec")
                    nc.vector.reciprocal(rec[:], a[:, 128:129])
                    nc.vector.tensor_scalar_mul(
                        ots[qs][:, h * 128:(h + 1) * 128],
                        a[:, 0:128], rec[:])
                    if qc_dma is not None:
                        q0 = qc_dma * QC + qs * 128
                        nc.sync.dma_start(out[q0:q0 + 128, :], ots[qs][:])

            # Filler schedule: qproj(qct, j) is split into dc-halves and
            # EDF-placed one-per-group across ALL groups so the PE has
            # independent work in every group where it would otherwise
            # stall on the Act engine's exp throughput. A half targeting
            # qct is available from group 4*(qct-1) (its x chunk's DMA
            # is issued then) and must land before group 4*qct + j.
            fillers = {}            # group -> [(qct, j, half)]
            gi = 0
            for qct in range(1, NQC):
                for j in range(NH):
                    for half in range(2):
                        avail = 4 * (qct - 1)
                        deadline = 4 * qct + j
                        g = max(avail, gi)
                        if g >= deadline:
                            g = deadline - 1
                        else:
                            gi = g + 1
                        fillers.setdefault(g, []).append((qct, j, half))

            qt_by = {0: qt_cur}
            xt_by = {}
            acc_by = {}
            for qc in range(NQC):
                if qc + 1 < NQC:
                    xt_by[qc + 1] = load_x_chunk(xqt, (qc + 1) * QC, QC)
                ots = [osb.tile([128, JB], F32, tag=f"ot{qs}",
                                name=f"ot{qs}") for qs in range(QC // 128)]
                et_prev = None
                for h in range(NH):
                    for (qct, j, half) in fillers.get(qc * NH + h, []):
                        acc, qt = emit_qproj_half(
                            j, xt_by[qct], half, acc_by.get((qct, j)))
                        acc_by[(qct, j)] = acc
                        if qt is not None:
                            qt_by.setdefault(qct, [None] * NH)[j] = qt
                    et_h = []
                    for k2 in range(NKT // 2):
                        s = sps.tile([128, 2 * QC], F32, tag="s", name="s")
                        for i in range(2):
                            nc.tensor.matmul(
                                s[:, i * QC:(i + 1) * QC],
                                K_T[h][:, (2 * k2 + i) * 128:
                                       (2 * k2 + i + 1) * 128],
                                qt_by[qc][h][:],
                                start=True, stop=True)
                        e = et_pool.tile([128, 2 * QC], BF16, tag=f"et{k2}",
                                         name=f"et{k2}")
                        nc.scalar.activation(
                            e[:], s[:], mybir.ActivationFunctionType.Exp,
                            scale=float(SCALE))
                        et_h.append(e)
                    if et_prev is not None:
                        emit_av(h - 1, et_prev, ots)
                    et_prev = et_h
                emit_av(NH - 1, et_prev, ots, qc_dma=qc)


_NC_CACHE = {}


def _get_nc(NP=1024, reps=1):
    if (NP, reps) not in _NC_CACHE:
        _NC_CACHE[(NP, reps)] = build_nc(NP, reps)
    return _NC_CACHE[(NP, reps)]


def _np_for_mask(attention_mask):
    n1 = int(max((attention_mask[b] != 0).sum() for b in range(B)))
    return int(min(L, max(QC, -(-n1 // QC) * QC)))


def _tile_xT(x):
    """x [rows, DIM] -> x^T dc-tiled [128, NDC, rows] (bf16)."""
    return np.ascontiguousarray(
        x.T.reshape(NDC, 128, x.shape[0]).transpose(1, 0, 2)).astype(BF16_NP)


def _tile_wT(w):
    """w [JB, DIM] -> w^T dc-tiled [128, NDC*JB] (bf16)."""
    return np.ascontiguousarray(
        w.T.reshape(NDC, 128, JB).transpose(1, 0, 2).reshape(
            128, NDC * JB)).astype(BF16_NP)


def make_in_maps(query_tensor, value_tensor, attention_mask, Wq, Wk, Wv):
    """Returns (in_maps, metas): metas[b] = packed row indices."""
    NP = _np_for_mask(attention_mask)
    in_maps, metas = [], []
    xqt_b, xvt_b = {}, {}
    for b in range(B):
        m = attention_mask[b]
        idx = np.flatnonzero(m != 0)
        xqp = np.zeros((NP, DIM), dtype=np.float32)
        xqp[:len(idx)] = query_tensor[b][idx] * m[idx, None]
        xqt_b[b] = _tile_xT(xqp)
        xvt_b[b] = _tile_xT(value_tensor[b])
        metas.append(idx)
    for c in range(N_CORES):
        b, g = divmod(c, 4)
        j0 = g * JB
        in_maps.append({
            "xqt": xqt_b[b],
            "xvt": xvt_b[b],
            "wqt": _tile_wT(Wq[j0:j0 + JB]),
            "wkt": _tile_wT(Wk[j0:j0 + JB]),
            "wvt": _tile_wT(Wv[j0:j0 + JB]),
        })
    return in_maps, metas


def assemble(results, value_tensor, attention_mask, Wv, metas):
    out = np.empty((B, L, DIM), dtype=np.float32)
    for b in range(B):
        masked = np.flatnonzero(attention_mask[b] == 0)
        if len(masked):
            vmean = value_tensor[b].mean(axis=0) @ Wv.T  # [DIM]
            out[b, masked, :] = vmean[None, :].astype(np.float32)
    for c in range(N_CORES):
        b, g = divmod(c, 4)
        idx = metas[b]
        out[b, idx, g * JB:(g + 1) * JB] = results[c]["out"][:len(idx)]
    return out


def kernel(query_tensor, value_tensor, attention_mask, Wq, Wk, Wv):
    args = [np.asarray(a) for a in (query_tensor, value_tensor,
                                    attention_mask, Wq, Wk, Wv)]
    nc = _get_nc(_np_for_mask(args[2]))
    in_maps, metas = make_in_maps(*args)
    res = run_bass_kernel_spmd(nc, in_maps, core_ids=list(range(N_CORES)))
    return assemble(res.results, args[1], args[2], args[5], metas)


# revision 21
# speedup vs baseline: 1.0917x; 1.0917x over previous
"""Multi-head attention (B=2, L=2048, DIM=2048, H=16) on 8 TRN2 NeuronCores.

Sharding: data-parallel over batch (2) x tensor-parallel over head groups (4).
Core c handles batch c//4, heads [4*(c%4), 4*(c%4)+4).

Host-side prep (not on the device critical path):
  - inputs are pre-transposed to feature-major layout and cast to bf16, so
    the device does zero PE transposes;
  - the multiplicative per-query mask is folded into xq, and only rows with
    nonzero mask are shipped (packed); masked rows' output is the uniform
    softmax = column-mean of v, reconstructed on the host.

Device math per core (head h, dh=128), all matmuls bf16 with fp32 PSUM:
  K_T[d, k] / V[k, j] from xvt chunks; Qt[d, q] from packed xqt chunks.
  S[k, q] = K_T . Qt;  E = exp(S / sqrt(128)) in bf16 (Act engine)
  attn[q, 0:128] / den[q, 128] from one matmul with a ones-column
  appended to V; out = attn * (1/den) per partition.
The q loop software-pipelines: Q-projection of chunk qc+1 and AV of head
h-1 are interleaved with scores/exp of head h so the PE never waits on
the Act engine's exp throughput.
"""

import sys

for p in ("/opt/trn_rl_repo", "/opt/pypackages"):
    if p not in sys.path:
        sys.path.insert(0, p)

import contextlib

import ml_dtypes
import numpy as np

import concourse.bacc as bacc
import concourse.mybir as mybir
import concourse.tile as tile
from concourse.bass_utils import run_bass_kernel_spmd

N_CORES = 8
B, L, DIM, H = 2, 2048, 2048, 16
JB = DIM // 4          # 512 output features per core (4 heads)
DH = 128               # head dim
NH = 4                 # heads per core
NDC = DIM // 128       # 16 contraction chunks
QC = 512               # q-chunk width (one PSUM bank of fp32)
SCALE = 1.0 / np.sqrt(DH)

F32 = mybir.dt.float32
BF16 = mybir.dt.bfloat16
BF16_NP = ml_dtypes.bfloat16


def build_nc(NP, reps=1):
    """DRAM layouts are host-pretiled so every stage is ONE batched DMA:
    x tensors as [128, NDC, cols] (partition-major dc tiling of x^T),
    W tensors as [128, NDC*JB] (partition-major dc tiling of W^T).
    reps>1 unrolls the whole kernel back-to-back inside one program
    (timing only: the axon bass_exec path allows one custom call per
    dispatch, so on-device unrolling is the only way to amortize
    dispatch overhead out of a measurement)."""
    nc = bacc.Bacc("TRN2", target_bir_lowering=False, debug=False,
                   num_devices=N_CORES)
    xqt = nc.dram_tensor("xqt", [128, NDC, NP], BF16,
                         kind="ExternalInput").ap()
    xvt = nc.dram_tensor("xvt", [128, NDC, L], BF16,
                         kind="ExternalInput").ap()
    wqt = nc.dram_tensor("wqt", [128, NDC * JB], BF16,
                         kind="ExternalInput").ap()
    wkt = nc.dram_tensor("wkt", [128, NDC * JB], BF16,
                         kind="ExternalInput").ap()
    wvt = nc.dram_tensor("wvt", [128, NDC * JB], BF16,
                         kind="ExternalInput").ap()
    out = nc.dram_tensor("out", [NP, JB], F32, kind="ExternalOutput").ap()

    with tile.TileContext(nc) as tc:
        for _ in range(reps):
            build_kernel(nc, tc, NP, xqt, xvt, wqt, wkt, wvt, out)
    nc.compile()
    return nc


def build_kernel(nc, tc, NP, xqt, xvt, wqt, wkt, wvt, out):
    NQC = NP // QC
    NKT = L // 128      # 16 k tiles

    with contextlib.ExitStack() as octx:
        w_pool = octx.enter_context(tc.tile_pool(name="w", bufs=1))
        kt_pool = octx.enter_context(tc.tile_pool(name="kt", bufs=1))
        v_pool = octx.enter_context(tc.tile_pool(name="v", bufs=1))
        xt_pool = octx.enter_context(tc.tile_pool(name="xt", bufs=2))
        qt_pool = octx.enter_context(tc.tile_pool(name="qt", bufs=2))
        pacc = octx.enter_context(
            tc.tile_pool(name="pacc", bufs=2, space="PSUM"))

        wq_a = w_pool.tile([128, NDC * JB], BF16, tag="wq", name="wq_a")
        wk_a = w_pool.tile([128, NDC * JB], BF16, tag="wk", name="wk_a")
        wv_a = w_pool.tile([128, NDC * JB], BF16, tag="wv", name="wv_a")
        K_T = [kt_pool.tile([128, L], BF16, tag=f"k{h}", name=f"kT{h}")
               for h in range(NH)]
        V = [v_pool.tile([128, NH * 129], BF16, tag=f"v{t}", name=f"vS{t}")
             for t in range(NKT)]

        for t in range(NKT):
            for h in range(NH):
                nc.vector.memset(V[t][:, h * 129 + 128:h * 129 + 129], 1.0)

        def load_x_chunk(x_dram, c0, w, splits=1):
            xt = xt_pool.tile([128, NDC * w], BF16, tag="xt", name="xt")
            if splits == 1:
                nc.sync.dma_start(xt[:], x_dram[:, :, c0:c0 + w])
            else:
                hd = NDC // splits
                for s in range(splits):
                    nc.sync.dma_start(
                        xt[:, s * hd * w:(s + 1) * hd * w],
                        x_dram[:, s * hd:(s + 1) * hd, c0:c0 + w])
            return xt

        def emit_qproj_half(j, xt, half, acc):
            """Half of head-j's Q projection (dc 8*half..8*half+8).
            Returns (acc, qt): qt is the drained bf16 tile after half 1."""
            if half == 0:
                acc = pacc.tile([128, QC], F32, tag="pacc", name="qacc")
            for dc in range(half * (NDC // 2), (half + 1) * (NDC // 2)):
                nc.tensor.matmul(
                    acc[:],
                    wq_a[:, dc * JB + j * 128:dc * JB + (j + 1) * 128],
                    xt[:, dc * QC:(dc + 1) * QC],
                    start=(dc == 0), stop=(dc == NDC - 1))
            if half == 0:
                return acc, None
            qt = qt_pool.tile([128, QC], BF16, tag=f"qt{j}", name=f"qt{j}")
            nc.vector.tensor_copy(qt[:], acc[:])
            return acc, qt

        def emit_qproj_j(j, xt):
            acc, _ = emit_qproj_half(j, xt, 0, None)
            _, qt = emit_qproj_half(j, xt, 1, acc)
            return qt

        # ---- prologue: Q projection of chunk 0 ----
        # DMA issue order front-loads exactly what the PE needs first;
        # wq/xq (and wk/xv) land as interleaved halves so the first
        # accumulation pass can start after ~1/4 of the transfer.
        qt_cur = None
        if NQC > 0:
            xt_q = xt_pool.tile([128, NDC * QC], BF16, tag="xt", name="xt")
            NS = 8
            wd, xd = NDC * JB // NS, NDC // NS
            for s in range(NS):
                nc.sync.dma_start(wq_a[:, s * wd:(s + 1) * wd],
                                  wqt[:, s * wd:(s + 1) * wd])
                nc.sync.dma_start(xt_q[:, s * xd * QC:(s + 1) * xd * QC],
                                  xqt[:, s * xd:(s + 1) * xd, 0:QC])
            qt_cur = [emit_qproj_j(j, xt_q) for j in range(NH)]
        else:
            nc.sync.dma_start(wq_a[:], wqt[:, :])

        # ---- K/V projections ----
        HW_ = NDC * JB // 2
        nc.sync.dma_start(wk_a[:, :HW_], wkt[:, :HW_])
        xt_v = load_x_chunk(xvt, 0, QC, splits=2)
        nc.sync.dma_start(wk_a[:, HW_:], wkt[:, HW_:])
        nc.sync.dma_start(wv_a[:], wvt[:, :])
        for lb in range(L // QC):
            for j in range(NH):
                acc = pacc.tile([128, QC], F32, tag="pacc", name="kacc")
                for dc in range(NDC):
                    nc.tensor.matmul(
                        acc[:],
                        wk_a[:, dc * JB + j * 128:dc * JB + (j + 1) * 128],
                        xt_v[:, dc * QC:(dc + 1) * QC],
                        start=(dc == 0), stop=(dc == NDC - 1))
                nc.vector.tensor_copy(K_T[j][:, lb * QC:(lb + 1) * QC],
                                      acc[:])
            for kt in range(QC // 128):
                acc = pacc.tile([128, JB], F32, tag="pacc", name="vacc")
                for dc in range(NDC):
                    nc.tensor.matmul(
                        acc[:],
                        xt_v[:, dc * QC + kt * 128:dc * QC + (kt + 1) * 128],
                        wv_a[:, dc * JB:(dc + 1) * JB],
                        start=(dc == 0), stop=(dc == NDC - 1))
                kc = lb * (QC // 128) + kt
                for h in range(NH):
                    nc.vector.tensor_copy(
                        V[kc][:, h * 129:h * 129 + 128],
                        acc[:, h * 128:(h + 1) * 128])
            if lb + 1 < L // QC:
                xt_v = load_x_chunk(xvt, (lb + 1) * QC, QC)

        # ---- attention over packed q chunks (sw-pipelined) ----
        with tc.tile_pool(name="et", bufs=2) as et_pool, \
             tc.tile_pool(name="s_ps", bufs=2, space="PSUM") as sps, \
             tc.tile_pool(name="a_ps", bufs=2, space="PSUM") as aps, \
             tc.tile_pool(name="o_sb", bufs=2) as osb, \
             tc.tile_pool(name="r_sb", bufs=4) as rsb:

            def emit_av(h, et_h, ots, qc_dma=None):
                for qs in range(QC // 128):
                    a = aps.tile([128, 129], F32, tag="a", name="a")
                    for kc in range(NKT):
                        nc.tensor.matmul(
                            a[:],
                            et_h[kc // 2][:, (kc % 2) * QC
                                          + qs * 128:(kc % 2) * QC
                                          + (qs + 1) * 128],
                            V[kc][:, h * 129:(h + 1) * 129],
                            start=(kc == 0), stop=(kc == NKT - 1))
                    rec = rsb.tile([128, 1], F32, tag="rec", name="rec")
                    nc.vector.reciprocal(rec[:], a[:, 128:129])
                    nc.vector.tensor_scalar_mul(
                        ots[qs][:, h * 128:(h + 1) * 128],
                        a[:, 0:128], rec[:])
                    if qc_dma is not None:
                        q0 = qc_dma * QC + qs * 128
                        nc.sync.dma_start(out[q0:q0 + 128, :], ots[qs][:])

            # Filler schedule: qproj(qct, j) is split into dc-halves and
            # EDF-placed one-per-group across ALL groups so the PE has
            # independent work in every group where it would otherwise
            # stall on the Act engine's exp throughput. A half targeting
            # qct is available from group 4*(qct-1) (its x chunk's DMA
            # is issued then) and must land before group 4*qct + j.
            fillers = {}            # group -> [(qct, j, half)]
            gi = 0
            for qct in range(1, NQC):
                for j in range(NH):
                    for half in range(2):
                        avail = 4 * (qct - 1)
                        deadline = 4 * qct + j
                        g = max(avail, gi)
                        if g >= deadline:
                            g = deadline - 1
                        else:
                            gi = g + 1
                        fillers.setdefault(g, []).append((qct, j, half))

            qt_by = {0: qt_cur}
            xt_by = {}
            acc_by = {}
            for qc in range(NQC):
                if qc + 1 < NQC:
                    xt_by[qc + 1] = load_x_chunk(xqt, (qc + 1) * QC, QC)
                ots = [osb.tile([128, JB], F32, tag=f"ot{qs}",
                                name=f"ot{qs}") for qs in range(QC // 128)]
                et_prev = None
                for h in range(NH):
                    for (qct, j, half) in fillers.get(qc * NH + h, []):
                        acc, qt = emit_qproj_half(
                            j, xt_by[qct], half, acc_by.get((qct, j)))
                        acc_by[(qct, j)] = acc
                        if qt is not None:
                            qt_by.setdefault(qct, [None] * NH)[j] = qt
                    et_h = []
                    for k2 in range(NKT // 2):
                        s = sps.tile([128, 2 * QC], F32, tag="s", name="s")
                        for i in range(2):
                            nc.tensor.matmul(
                                s[:, i * QC:(i + 1) * QC],
                                K_T[h][:, (2 * k2 + i) * 128:
                                       (2 * k2 + i + 1) * 128],
                                qt_by[qc][h][:],
                                start=True, stop=True)
                        e = et_pool.tile([128, 2 * QC], BF16, tag=f"et{k2}",
                                         name=f"et{k2}")
                        nc.scalar.activation(
                            e[:], s[:], mybir.ActivationFunctionType.Exp,
                            scale=float(SCALE))
                        et_h.append(e)
                    if et_prev is not None:
                        emit_av(h - 1, et_prev, ots)
                    et_prev = et_h
                emit_av(NH - 1, et_prev, ots, qc_dma=qc)


_NC_CACHE = {}


def _get_nc(NP=1024, reps=1):
    if (NP, reps) not in _NC_CACHE:
        _NC_CACHE[(NP, reps)] = build_nc(NP, reps)
    return _NC_CACHE[(NP, reps)]


def _np_for_mask(attention_mask):
    n1 = int(max((attention_mask[b] != 0).sum() for b in range(B)))
    return int(min(L, max(QC, -(-n1 // QC) * QC)))


def _tile_xT(x):
    """x [rows, DIM] -> x^T dc-tiled [128, NDC, rows] (bf16)."""
    return np.ascontiguousarray(
        x.T.reshape(NDC, 128, x.shape[0]).transpose(1, 0, 2)).astype(BF16_NP)


def _tile_wT(w):
    """w [JB, DIM] -> w^T dc-tiled [128, NDC*JB] (bf16)."""
    return np.ascontiguousarray(
        w.T.reshape(NDC, 128, JB).transpose(1, 0, 2).reshape(
            128, NDC * JB)).astype(BF16_NP)


def make_in_maps(query_tensor, value_tensor, attention_mask, Wq, Wk, Wv):
    """Returns (in_maps, metas): metas[b] = packed row indices."""
    NP = _np_for_mask(attention_mask)
    in_maps, metas = [], []
    xqt_b, xvt_b = {}, {}
    for b in range(B):
        m = attention_mask[b]
        idx = np.flatnonzero(m != 0)
        xqp = np.zeros((NP, DIM), dtype=np.float32)
        xqp[:len(idx)] = query_tensor[b][idx] * m[idx, None]
        xqt_b[b] = _tile_xT(xqp)
        xvt_b[b] = _tile_xT(value_tensor[b])
        metas.append(idx)
    for c in range(N_CORES):
        b, g = divmod(c, 4)
        j0 = g * JB
        in_maps.append({
            "xqt": xqt_b[b],
            "xvt": xvt_b[b],
            "wqt": _tile_wT(Wq[j0:j0 + JB]),
            "wkt": _tile_wT(Wk[j0:j0 + JB]),
            "wvt": _tile_wT(Wv[j0:j0 + JB]),
        })
    return in_maps, metas


def assemble(results, value_tensor, attention_mask, Wv, metas):
    out = np.empty((B, L, DIM), dtype=np.float32)
    for b in range(B):
        masked = np.flatnonzero(attention_mask[b] == 0)
        if len(masked):
            vmean = value_tensor[b].mean(axis=0) @ Wv.T  # [DIM]
            out[b, masked, :] = vmean[None, :].astype(np.float32)
    for c in range(N_CORES):
        b, g = divmod(c, 4)
        idx = metas[b]
        out[b, idx, g * JB:(g + 1) * JB] = results[c]["out"][:len(idx)]
    return out


def kernel(query_tensor, value_tensor, attention_mask, Wq, Wk, Wv):
    args = [np.asarray(a) for a in (query_tensor, value_tensor,
                                    attention_mask, Wq, Wk, Wv)]
    nc = _get_nc(_np_for_mask(args[2]))
    in_maps, metas = make_in_maps(*args)
    last = None
    for _ in range(3):      # the axon tunnel throws transient errors
        try:
            res = run_bass_kernel_spmd(nc, in_maps,
                                       core_ids=list(range(N_CORES)))
            break
        except Exception as e:  # noqa: BLE001
            last = e
    else:
        raise last
    return assemble(res.results, args[1], args[2], args[5], metas)
